# revision 15
# baseline (speedup 1.0000x reference)
import sys
import numpy as np

sys.path.insert(0, "/opt/trn_rl_repo")

import concourse.bass as bass
import concourse.bacc as bacc
import concourse.mybir as mybir
from concourse.tile import TileContext
from concourse import masks
from concourse.bass_utils import run_bass_kernel_spmd

F32 = mybir.dt.float32
I32 = mybir.dt.int32
U8 = mybir.dt.uint8
AX = mybir.AxisListType.X
OP = mybir.AluOpType
ACTF = mybir.ActivationFunctionType

BS = 2            # batches per core
NB = 16           # full batch
NCORES = 8
NA = 8400
NCLS = 80
NG = 64
W = 336           # stage-1 anchor tile width
NT = NA // W      # 25
CH = 128          # stage-2 chunk width
NCH = (NA + CH - 1) // CH  # 66
K4 = 40.0         # TOP_K * 4
CV = 4.0 / (np.pi ** 2)


def build():
    nc = bacc.Bacc()
    score = nc.declare_dram_parameter("score", [BS, NA, NCLS], F32, isOutput=False)
    p_box = nc.declare_dram_parameter("p_box", [BS, NA, 4], F32, isOutput=False)
    anchors = nc.declare_dram_parameter("anchors", [NA, 2], F32, isOutput=False)
    gt_labels = nc.declare_dram_parameter("gt_labels", [BS, NG, 1], I32, isOutput=False)
    gt_box = nc.declare_dram_parameter("gt_box", [BS, NG, 4], F32, isOutput=False)
    maskp = nc.declare_dram_parameter("mask", [BS, NG, 1], F32, isOutput=False)
    out = nc.declare_dram_parameter("out", [BS, NA, 85], F32, isOutput=True)

    with TileContext(nc) as tc:
        with (
            tc.tile_pool(name="big", bufs=1) as big,
            tc.tile_pool(name="sm", bufs=1) as sm,
            tc.tile_pool(name="tmp", bufs=14) as tp,
            tc.tile_pool(name="ps", bufs=6, space="PSUM") as psp,
        ):
            metric = big.tile([128, NA], F32, tag="metric")
            overlaps = big.tile([128, NA], F32, tag="overlaps")
            sel = big.tile([128, NA], F32, tag="sel")
            scr = big.tile([128, NA], F32, tag="scr")
            mgts = big.tile([128, NA], U8, tag="mgts")

            ident = sm.tile([128, 128], F32, tag="ident")
            masks.make_identity(nc, ident[:, :])
            ones1 = sm.tile([1, 128], F32, tag="ones1")
            nc.gpsimd.memset(ones1[:, :], 1.0)
            ones64 = sm.tile([128, 1], F32, tag="ones64")
            nc.gpsimd.memset(ones64[:, :], 1.0)

            # ---- gt-side scalars, packed 2 batches x 64 gts = 128 partitions
            gtc = sm.tile([128, 4], F32, tag="gtc")
            nc.sync.dma_start(out=gtc[:, :], in_=gt_box.rearrange("b n f -> (b n) f"))
            lrow_f = []
            for b in range(2):
                li = sm.tile([1, NG], I32, tag=f"li{b}")
                nc.gpsimd.dma_start(out=li[:, :], in_=gt_labels[b, :, 0:1].rearrange("n o -> o n"))
                lf = sm.tile([1, NG], F32, tag=f"lf{b}")
                nc.vector.tensor_copy(lf[:, :], li[:, :])
                lrow_f.append(lf)
            valid = sm.tile([128, 1], F32, tag="valid")
            nc.sync.dma_start(out=valid[:, :], in_=maskp.rearrange("b n f -> (b n) f"))

            x1s, y1s, x2s, y2s = (gtc[:, k:k + 1] for k in range(4))
            gs = sm.tile([128, 8], F32, tag="gs")
            w1, h1e, atan1, area1, sumx1, sumy1, rh1, rat1 = (gs[:, k:k + 1] for k in range(8))
            nc.any.tensor_tensor(out=w1, in0=x2s, in1=x1s, op=OP.subtract)
            nc.any.tensor_tensor(out=h1e, in0=y2s, in1=y1s, op=OP.subtract)
            nc.any.tensor_scalar(out=h1e, in0=h1e, scalar1=1e-7, scalar2=None, op0=OP.add)
            nc.vector.reciprocal(out=rh1, in_=h1e)
            nc.any.tensor_tensor(out=rat1, in0=w1, in1=rh1, op=OP.mult)
            nc.scalar.activation(atan1, rat1, ACTF.Arctan)
            nc.any.tensor_tensor(out=area1, in0=w1, in1=h1e, op=OP.mult)
            nc.any.tensor_tensor(out=sumx1, in0=x1s, in1=x2s, op=OP.add)
            nc.any.tensor_tensor(out=sumy1, in0=y1s, in1=y2s, op=OP.add)

            # ---- one-hot(labels)^T [80, 64] per batch for the class gather
            iotac80 = sm.tile([80, 1], F32, tag="iotac80")
            nc.gpsimd.iota(iotac80[:, :], pattern=[[0, 1]], base=0, channel_multiplier=1,
                           allow_small_or_imprecise_dtypes=True)
            onehotT = []
            for b in range(2):
                pb = psp.tile([128, W], F32, tag="ps")
                nc.tensor.matmul(pb[0:80, 0:NG], ones1[0:1, 0:80], lrow_f[b][:, :])
                oh = sm.tile([80, NG], F32, tag=f"oht{b}")
                nc.any.tensor_scalar(out=oh[:, :], in0=pb[0:80, 0:NG], scalar1=iotac80[:, :],
                                     scalar2=None, op0=OP.is_equal)
                onehotT.append(oh)

            # ================= stage 1: metric / overlaps / mgts =================
            tc.strict_bb_all_engine_barrier()
            for t in range(NT):
                a0 = t * W
                # broadcast pred-side rows to 128 partitions via PE outer product
                axb = tp.tile([128, W], F32, tag="t")
                ayb = tp.tile([128, W], F32, tag="t")
                for k, dst in ((0, axb), (1, ayb)):
                    arow = tp.tile([1, W], F32, tag="r")
                    nc.gpsimd.dma_start(out=arow[:, :],
                                      in_=anchors[a0:a0 + W, k:k + 1].rearrange("a o -> o a"))
                    pb = psp.tile([128, W], F32, tag="ps")
                    nc.tensor.matmul(pb[0:128, 0:W], ones1[0:1, 0:128], arow[:, :])
                    nc.vector.tensor_copy(dst[:, :], pb[0:128, 0:W])
                coords = []
                for k in range(4):
                    dst = tp.tile([128, W], F32, tag="t")
                    for b in range(2):
                        prow = tp.tile([1, W], F32, tag="r")
                        nc.gpsimd.dma_start(out=prow[:, :],
                                          in_=p_box[b, a0:a0 + W, k:k + 1].rearrange("a o -> o a"))
                        pb = psp.tile([128, W], F32, tag="ps")
                        nc.tensor.matmul(pb[0:64, 0:W], ones1[0:1, 0:64], prow[:, :])
                        nc.vector.tensor_copy(dst[64 * b:64 * b + 64, :], pb[0:64, 0:W])
                    coords.append(dst)
                x1p, y1p, x2p, y2p = coords

                # gathered scores: transpose score chunks then one-hot matmul
                gath = tp.tile([128, W], F32, tag="t")
                for b in range(2):
                    sct = sm.tile([80, W], F32, tag=f"sct{b}")
                    off = 0
                    while off < W:
                        cw = min(128, W - off)
                        st = tp.tile([128, NCLS], F32, tag="st")
                        nc.sync.dma_start(out=st[0:cw, :], in_=score[b, a0 + off:a0 + off + cw, :])
                        pb = psp.tile([128, W], F32, tag="ps")
                        nc.tensor.transpose(pb[0:NCLS, 0:cw], st[0:cw, 0:NCLS], ident[0:cw, 0:cw])
                        nc.vector.tensor_copy(sct[:, off:off + cw], pb[0:NCLS, 0:cw])
                        off += cw
                    pb = psp.tile([128, W], F32, tag="ps")
                    nc.tensor.matmul(pb[0:64, 0:W], onehotT[b][:, :], sct[:, :])
                    nc.vector.tensor_copy(gath[64 * b:64 * b + 64, :], pb[0:64, 0:W])

                def ts(dst, in0, s1, op0, s2=None, op1=OP.bypass):
                    nc.any.tensor_scalar(out=dst, in0=in0, scalar1=s1, scalar2=s2, op0=op0, op1=op1)

                def tt(dst, a, b_, op):
                    nc.any.tensor_tensor(out=dst, in0=a, in1=b_, op=op)

                def tmp():
                    return tp.tile([128, W], F32, tag="t", name="tw")[:, :]

                MG = mgts[:, a0:a0 + W]
                # mask_in_gts & valid
                t1 = tmp(); ts(t1, axb[:, :], x1s, OP.subtract)
                t2 = tmp(); ts(t2, ayb[:, :], y1s, OP.subtract)
                tt(t1, t1, t2, OP.min)
                ts(t2, axb[:, :], x2s, OP.subtract)
                t3 = tmp(); ts(t3, ayb[:, :], y2s, OP.subtract)
                tt(t2, t2, t3, OP.max)
                ts(t2, t2, -1.0, OP.mult)
                tt(t1, t1, t2, OP.min)
                nc.any.tensor_scalar(out=MG, in0=t1, scalar1=1e-9, scalar2=valid[:, :],
                                     op0=OP.is_gt, op1=OP.mult)

                # pred-side derived vectors
                w2 = tmp(); tt(w2, x2p[:, :], x1p[:, :], OP.subtract)
                h2 = tmp(); tt(h2, y2p[:, :], y1p[:, :], OP.subtract)
                ts(h2, h2, 1e-7, OP.add)
                rr = tmp(); nc.vector.reciprocal(out=rr, in_=h2)
                tt(rr, w2, rr, OP.mult)
                at2 = tmp(); nc.scalar.activation(at2, rr, ACTF.Arctan)

                # intersection / union / iou
                a1 = tmp(); ts(a1, x2p[:, :], x2s, OP.min)
                a2 = tmp(); ts(a2, x1p[:, :], x1s, OP.max)
                tt(a1, a1, a2, OP.subtract)
                ts(a1, a1, 0.0, OP.max)
                ts(a2, y2p[:, :], y2s, OP.min)
                b2 = tmp(); ts(b2, y1p[:, :], y1s, OP.max)
                tt(a2, a2, b2, OP.subtract)
                ts(a2, a2, 0.0, OP.max)
                inter = a1; tt(inter, a1, a2, OP.mult)
                u = a2; tt(u, w2, h2, OP.mult)
                nc.any.tensor_scalar(out=u, in0=u, scalar1=area1, scalar2=1e-7, op0=OP.add, op1=OP.add)
                tt(u, u, inter, OP.subtract)
                ru = b2; nc.vector.reciprocal(out=ru, in_=u)
                iou = u; tt(iou, inter, ru, OP.mult)

                # enclosing box diag
                c1 = inter  # reuse
                ts(c1, x2p[:, :], x2s, OP.max)
                c2 = ru
                ts(c2, x1p[:, :], x1s, OP.min)
                tt(c1, c1, c2, OP.subtract)
                tt(c1, c1, c1, OP.mult)
                ts(c2, y2p[:, :], y2s, OP.max)
                d2 = tmp(); ts(d2, y1p[:, :], y1s, OP.min)
                tt(c2, c2, d2, OP.subtract)
                tt(c2, c2, c2, OP.mult)
                tt(c1, c1, c2, OP.add)
                ts(c1, c1, 1e-7, OP.add)
                rc2 = c2; nc.vector.reciprocal(out=rc2, in_=c1)

                # rho2/4
                dx = c1
                tt(dx, x1p[:, :], x2p[:, :], OP.add)
                ts(dx, dx, sumx1, OP.subtract)
                tt(dx, dx, dx, OP.mult)
                dy = d2
                tt(dy, y1p[:, :], y2p[:, :], OP.add)
                ts(dy, dy, sumy1, OP.subtract)
                tt(dy, dy, dy, OP.mult)
                tt(dx, dx, dy, OP.add)
                ts(dx, dx, 0.25, OP.mult)
                rho4 = dx

                # v and v*alpha
                v = dy
                ts(v, at2, atan1, OP.subtract)
                tt(v, v, v, OP.mult)
                ts(v, v, CV, OP.mult)
                va = at2
                tt(va, v, iou, OP.subtract)
                ts(va, va, 1.0 + 1e-7, OP.add)
                rva = w2
                nc.vector.reciprocal(out=rva, in_=va)
                tt(va, v, rva, OP.mult)
                tt(va, va, v, OP.mult)

                # ov = iou - rho4*rc2 - va ; clip; mask
                tt(rho4, rho4, rc2, OP.mult)
                tt(rho4, rho4, va, OP.add)
                OV = overlaps[:, a0:a0 + W]
                tt(OV, iou, rho4, OP.subtract)
                ts(OV, OV, 0.0, OP.max)
                tt(OV, OV, MG, OP.mult)

                # metric = sqrt(gathered) * ov^6
                sq = h2
                nc.scalar.activation(sq, gath[:, :], ACTF.Sqrt)
                o2 = rr
                tt(o2, OV, OV, OP.mult)
                o4 = v
                tt(o4, o2, o2, OP.mult)
                tt(o4, o4, o2, OP.mult)
                ME = metric[:, a0:a0 + W]
                tt(ME, sq, o4, OP.mult)
                if t % 2 == 1:
                    tc.strict_bb_all_engine_barrier()

            # ===== top-k: remove max value-class from a WORKING COPY 10x, then
            # ===== select from the pristine metric against the exact threshold
            tc.strict_bb_all_engine_barrier()
            mjs = sm.tile([128, 10], F32, tag="mjs")
            cnts = sm.tile([128, 10], F32, tag="cnts")
            gate = sm.tile([128, 1], F32, tag="gate")
            mjp1 = sm.tile([128, 1], F32, tag="mjp1")
            nc.vector.tensor_copy(scr[:, :], metric[:, :])
            for j in range(10):
                mj = mjs[:, j:j + 1]
                nc.vector.tensor_reduce(out=mj, in_=scr[:, :], axis=AX, op=OP.max)
                nc.any.tensor_scalar(out=gate[:, :], in0=mj, scalar1=0.0, scalar2=None, op0=OP.is_ge)
                nc.any.tensor_scalar(out=mjp1[:, :], in0=mj, scalar1=1.0, scalar2=gate[:, :],
                                     op0=OP.add, op1=OP.mult)
                nc.any.tensor_scalar(out=sel[:, :], in0=scr[:, :], scalar1=mj,
                                     scalar2=gate[:, :], op0=OP.is_equal, op1=OP.mult)
                nc.vector.tensor_reduce(out=cnts[:, j:j + 1], in_=sel[:, :], axis=AX, op=OP.add)
                # removed positions become exactly -1 (x - x - 1)
                nc.any.tensor_scalar(out=sel[:, :], in0=sel[:, :], scalar1=mjp1[:, :],
                                     scalar2=None, op0=OP.mult)
                nc.any.tensor_tensor(out=scr[:, :], in0=scr[:, :], in1=sel[:, :], op=OP.subtract)

            cum = sm.tile([128, 10], F32, tag="cum")
            nc.vector.tensor_tensor_scan(out=cum[:, :], data0=cnts[:, :], data1=cnts[:, :],
                                         initial=0.0, op0=OP.add, op1=OP.bypass)
            excl = sm.tile([128, 10], F32, tag="excl")
            nc.any.tensor_tensor(out=excl[:, :], in0=cum[:, :], in1=cnts[:, :], op=OP.subtract)
            pick = sm.tile([128, 10], F32, tag="pick")
            nc.any.tensor_scalar(out=pick[:, :], in0=excl[:, :], scalar1=10.0, scalar2=None, op0=OP.is_lt)
            nc.any.tensor_scalar(out=cum[:, :], in0=cum[:, :], scalar1=10.0, scalar2=None, op0=OP.is_ge)
            nc.any.tensor_tensor(out=pick[:, :], in0=pick[:, :], in1=cum[:, :], op=OP.mult)
            Tv = sm.tile([128, 1], F32, tag="Tv")
            nc.any.tensor_tensor(out=cum[:, :], in0=pick[:, :], in1=mjs[:, :], op=OP.mult)
            nc.vector.tensor_reduce(out=Tv[:, :], in_=cum[:, :], axis=AX, op=OP.add)
            mallow = sm.tile([128, 1], F32, tag="mallow")
            nc.any.tensor_tensor(out=cum[:, :], in0=pick[:, :], in1=excl[:, :], op=OP.mult)
            nc.vector.tensor_reduce(out=mallow[:, :], in_=cum[:, :], axis=AX, op=OP.add)
            nc.any.tensor_scalar(out=mallow[:, :], in0=mallow[:, :], scalar1=-1.0, scalar2=10.0,
                                 op0=OP.mult, op1=OP.add)

            # exact selection from pristine metric: v > T, or v == T with tie-rank <= m
            nc.any.tensor_scalar(out=sel[:, :], in0=metric[:, :], scalar1=Tv[:, :], scalar2=None,
                                 op0=OP.is_equal)
            nc.vector.tensor_tensor_scan(out=scr[:, :], data0=sel[:, :], data1=sel[:, :],
                                         initial=0.0, op0=OP.add, op1=OP.bypass)
            nc.any.tensor_scalar(out=scr[:, :], in0=scr[:, :], scalar1=mallow[:, :], scalar2=None,
                                 op0=OP.is_le)
            nc.any.tensor_tensor(out=scr[:, :], in0=scr[:, :], in1=sel[:, :], op=OP.mult)
            nc.any.tensor_scalar(out=sel[:, :], in0=metric[:, :], scalar1=Tv[:, :], scalar2=None,
                                 op0=OP.is_gt)
            nc.any.tensor_tensor(out=sel[:, :], in0=sel[:, :], in1=scr[:, :], op=OP.add)
            nc.any.tensor_tensor(out=sel[:, :], in0=sel[:, :], in1=mgts[:, :], op=OP.mult)

            # ====== stage 2: per-anchor-chunk resolution in transposed layout ======
            tc.strict_bb_all_engine_barrier()
            iota64mb = sm.tile([128, 64], F32, tag="i64")
            nc.gpsimd.iota(iota64mb[:, :], pattern=[[1, 64]], base=-16384, channel_multiplier=0,
                           allow_small_or_imprecise_dtypes=True)
            iota80b = sm.tile([128, 80], F32, tag="i80")
            nc.gpsimd.iota(iota80b[:, :], pattern=[[1, 80]], base=0, channel_multiplier=0,
                           allow_small_or_imprecise_dtypes=True)
            # broadcast gt tables along free dim: cols 0:64 = batch0, 64:128 = batch1
            gtabs = []
            for k in range(4):
                gtab = sm.tile([128, 128], F32, tag=f"gtab{k}")
                for b in range(2):
                    grow = sm.tile([1, NG], F32, tag="grow")
                    nc.gpsimd.dma_start(out=grow[:, :],
                                      in_=gt_box[b, :, k:k + 1].rearrange("n o -> o n"))
                    pb = psp.tile([128, W], F32, tag="ps")
                    nc.tensor.matmul(pb[0:128, 0:NG], ones1[0:1, 0:128], grow[:, :])
                    nc.vector.tensor_copy(gtab[:, 64 * b:64 * b + 64], pb[0:128, 0:NG])
                gtabs.append(gtab)
            lblt = sm.tile([128, 128], F32, tag="lblt")
            for b in range(2):
                pb = psp.tile([128, W], F32, tag="ps")
                nc.tensor.matmul(pb[0:128, 0:NG], ones1[0:1, 0:128], lrow_f[b][:, :])
                nc.vector.tensor_copy(lblt[:, 64 * b:64 * b + 64], pb[0:128, 0:NG])

            fg2all = sm.tile([128, 2 * NCH], F32, tag="fg2all")
            lblall = sm.tile([128, 2 * NCH], F32, tag="lblall")

            for c in range(NCH):
                a0 = c * CH
                cw = min(CH, NA - a0)
                mpT = tp.tile([128, 128], F32, tag="t")
                ovT = tp.tile([128, 128], F32, tag="t")
                for b in range(2):
                    idb = ident[64 * b:64 * b + 64, 64 * b:64 * b + 64]
                    pb = psp.tile([128, W], F32, tag="ps")
                    nc.tensor.transpose(pb[0:cw, 0:64], sel[64 * b:64 * b + 64, a0:a0 + cw], idb)
                    nc.vector.tensor_copy(mpT[0:cw, 64 * b:64 * b + 64], pb[0:cw, 0:64])
                    pb2 = psp.tile([128, W], F32, tag="ps")
                    nc.tensor.transpose(pb2[0:cw, 0:64], overlaps[64 * b:64 * b + 64, a0:a0 + cw], idb)
                    nc.vector.tensor_copy(ovT[0:cw, 64 * b:64 * b + 64], pb2[0:cw, 0:64])
                for b in range(2):
                    S = slice(64 * b, 64 * b + 64)
                    fg = tp.tile([128, 1], F32, tag="s1")
                    nc.vector.tensor_reduce(out=fg[0:cw, :], in_=mpT[0:cw, S], axis=AX, op=OP.add)
                    rmx = tp.tile([128, 1], F32, tag="s1")
                    nc.vector.tensor_reduce(out=rmx[0:cw, :], in_=ovT[0:cw, S], axis=AX, op=OP.max)
                    e = tp.tile([128, 64], F32, tag="t")
                    nc.any.tensor_scalar(out=e[0:cw, :], in0=ovT[0:cw, S], scalar1=rmx[0:cw, :],
                                         scalar2=None, op0=OP.is_equal)
                    nc.any.tensor_tensor(out=e[0:cw, :], in0=e[0:cw, :], in1=iota64mb[0:cw, :], op=OP.mult)
                    rmn = tp.tile([128, 1], F32, tag="s1")
                    nc.vector.tensor_reduce(out=rmn[0:cw, :], in_=e[0:cw, :], axis=AX, op=OP.min)
                    oh = tp.tile([128, 64], F32, tag="t")
                    nc.any.tensor_scalar(out=oh[0:cw, :], in0=e[0:cw, :], scalar1=rmn[0:cw, :],
                                         scalar2=None, op0=OP.is_equal)
                    g = tp.tile([128, 1], F32, tag="s1")
                    nc.any.tensor_scalar(out=g[0:cw, :], in0=fg[0:cw, :], scalar1=1.0, scalar2=None,
                                         op0=OP.is_gt)
                    nc.any.tensor_tensor(out=oh[0:cw, :], in0=oh[0:cw, :], in1=mpT[0:cw, S], op=OP.subtract)
                    nc.any.tensor_scalar(out=oh[0:cw, :], in0=oh[0:cw, :], scalar1=g[0:cw, :],
                                         scalar2=None, op0=OP.mult)
                    nc.any.tensor_tensor(out=mpT[0:cw, S], in0=mpT[0:cw, S], in1=oh[0:cw, :], op=OP.add)
                    nc.vector.tensor_reduce(out=fg2all[0:cw, 2 * c + b:2 * c + b + 1],
                                            in_=mpT[0:cw, S], axis=AX, op=OP.add)
                    # gt_idx one-hot (argmax of resolved mask over gts)
                    nc.vector.tensor_reduce(out=rmx[0:cw, :], in_=mpT[0:cw, S], axis=AX, op=OP.max)
                    nc.any.tensor_scalar(out=e[0:cw, :], in0=mpT[0:cw, S], scalar1=rmx[0:cw, :],
                                         scalar2=None, op0=OP.is_equal)
                    nc.any.tensor_tensor(out=e[0:cw, :], in0=e[0:cw, :], in1=iota64mb[0:cw, :], op=OP.mult)
                    nc.vector.tensor_reduce(out=rmn[0:cw, :], in_=e[0:cw, :], axis=AX, op=OP.min)
                    nc.any.tensor_scalar(out=e[0:cw, :], in0=e[0:cw, :], scalar1=rmn[0:cw, :],
                                         scalar2=None, op0=OP.is_equal)
                    # gather label + bbox via one-hot
                    ob = tp.tile([128, 64], F32, tag="t")
                    nc.any.tensor_tensor(out=ob[0:cw, :], in0=e[0:cw, :], in1=lblt[0:cw, S], op=OP.mult)
                    nc.vector.tensor_reduce(out=lblall[0:cw, 2 * c + b:2 * c + b + 1],
                                            in_=ob[0:cw, :], axis=AX, op=OP.add)
                    outsb = tp.tile([128, 5], F32, tag="o5")
                    for k in range(4):
                        nc.any.tensor_tensor(out=ob[0:cw, :], in0=e[0:cw, :], in1=gtabs[k][0:cw, S],
                                             op=OP.mult)
                        nc.vector.tensor_reduce(out=outsb[0:cw, k:k + 1], in_=ob[0:cw, :], axis=AX,
                                                op=OP.add)
                    nc.any.tensor_scalar(out=outsb[0:cw, 4:5], in0=fg2all[0:cw, 2 * c + b:2 * c + b + 1],
                                         scalar1=0.0, scalar2=None, op0=OP.is_gt)
                    nc.sync.dma_start(out=out[b, a0:a0 + cw, 0:4], in_=outsb[0:cw, 0:4])
                    nc.sync.dma_start(out=out[b, a0:a0 + cw, 84:85], in_=outsb[0:cw, 4:5])
                # write resolved mask back to gt-layout
                for b in range(2):
                    pb = psp.tile([128, W], F32, tag="ps")
                    nc.tensor.transpose(pb[0:64, 0:cw], mpT[0:cw, 64 * b:64 * b + 64],
                                        ident[0:cw, 0:cw])
                    nc.vector.tensor_copy(sel[64 * b:64 * b + 64, a0:a0 + cw], pb[0:64, 0:cw])
                if c % 4 == 3:
                    tc.strict_bb_all_engine_barrier()

            # ====== stage 3: normalization factors and per-anchor score scale ======
            tc.strict_bb_all_engine_barrier()
            nc.any.tensor_tensor(out=scr[:, :], in0=metric[:, :], in1=sel[:, :], op=OP.mult)
            posm = sm.tile([128, 1], F32, tag="posm")
            nc.vector.tensor_reduce(out=posm[:, :], in_=scr[:, :], axis=AX, op=OP.max)
            nc.any.tensor_tensor(out=overlaps[:, :], in0=overlaps[:, :], in1=sel[:, :], op=OP.mult)
            poso = sm.tile([128, 1], F32, tag="poso")
            nc.vector.tensor_reduce(out=poso[:, :], in_=overlaps[:, :], axis=AX, op=OP.max)
            nc.any.tensor_scalar(out=posm[:, :], in0=posm[:, :], scalar1=1e-9, scalar2=None, op0=OP.add)
            rpm = sm.tile([128, 1], F32, tag="rpm")
            nc.vector.reciprocal(out=rpm[:, :], in_=posm[:, :])
            nc.any.tensor_tensor(out=rpm[:, :], in0=poso[:, :], in1=rpm[:, :], op=OP.mult)
            nc.any.tensor_scalar(out=scr[:, :], in0=scr[:, :], scalar1=rpm[:, :], scalar2=None,
                                 op0=OP.mult)

            # ====== stage 4: final scores ======
            for c in range(NCH):
                a0 = c * CH
                cw = min(CH, NA - a0)
                for b in range(2):
                    spb = psp.tile([128, W], F32, tag="ps")
                    nc.tensor.matmul(spb[0:cw, 0:1], scr[64 * b:64 * b + 64, a0:a0 + cw],
                                     ones64[64 * b:64 * b + 64, :])
                    ssb = tp.tile([128, 1], F32, tag="s1")
                    nc.vector.tensor_copy(ssb[0:cw, :], spb[0:cw, 0:1])
                    f2 = tp.tile([128, 1], F32, tag="s1")
                    nc.any.tensor_scalar(out=f2[0:cw, :], in0=fg2all[0:cw, 2 * c + b:2 * c + b + 1],
                                         scalar1=0.0, scalar2=None, op0=OP.is_gt)
                    osc = tp.tile([128, 80], F32, tag="t")
                    nc.any.tensor_scalar(out=osc[0:cw, :], in0=iota80b[0:cw, :],
                                         scalar1=lblall[0:cw, 2 * c + b:2 * c + b + 1],
                                         scalar2=f2[0:cw, :], op0=OP.is_equal, op1=OP.mult)
                    nc.any.tensor_scalar(out=osc[0:cw, :], in0=osc[0:cw, :], scalar1=ssb[0:cw, :],
                                         scalar2=None, op0=OP.mult)
                    nc.sync.dma_start(out=out[b, a0:a0 + cw, 4:84], in_=osc[0:cw, :])
                if c % 8 == 7:
                    tc.strict_bb_all_engine_barrier()
    nc.compile()
    return nc


_NC_CACHE = None


def kernel(score, p_box, anchors, gt_labels, gt_box, mask):
    global _NC_CACHE
    if _NC_CACHE is None:
        _NC_CACHE = build()
    nc = _NC_CACHE
    in_maps = []
    for c in range(NCORES):
        s = slice(c * BS, (c + 1) * BS)
        in_maps.append({
            "score": np.ascontiguousarray(score[s], dtype=np.float32),
            "p_box": np.ascontiguousarray(p_box[s], dtype=np.float32),
            "anchors": np.ascontiguousarray(anchors, dtype=np.float32),
            "gt_labels": np.ascontiguousarray(gt_labels[s], dtype=np.int32),
            "gt_box": np.ascontiguousarray(gt_box[s], dtype=np.float32),
            "mask": np.ascontiguousarray(mask[s], dtype=np.float32),
        })
    res = run_bass_kernel_spmd(nc, in_maps, list(range(NCORES)))
    outs = [np.asarray(res.results[c]["out"]) for c in range(NCORES)]
    full = np.concatenate(outs, axis=0)  # [16, 8400, 85]
    tb = np.ascontiguousarray(full[..., 0:4], dtype=np.float32)
    tsc = np.ascontiguousarray(full[..., 4:84], dtype=np.float32)
    fg = full[..., 84] > 0.5
    return tb, tsc, fg


# revision 17
# speedup vs baseline: 1.1885x; 1.1885x over previous
import sys
import numpy as np

sys.path.insert(0, "/opt/trn_rl_repo")

import concourse.bass as bass
import concourse.bacc as bacc
import concourse.mybir as mybir
from concourse.tile import TileContext
from concourse import masks
from concourse.bass_utils import run_bass_kernel_spmd

F32 = mybir.dt.float32
I32 = mybir.dt.int32
U8 = mybir.dt.uint8
AX = mybir.AxisListType.X
OP = mybir.AluOpType
ACTF = mybir.ActivationFunctionType

BS = 2            # batches per core
NB = 16           # full batch
NCORES = 8
NA = 8400
NCLS = 80
NG = 64
W = 336           # stage-1 anchor tile width
NT = NA // W      # 25
CH = 128          # stage-2 chunk width
NCH = (NA + CH - 1) // CH  # 66
K4 = 40.0         # TOP_K * 4
CV = 4.0 / (np.pi ** 2)


def build():
    nc = bacc.Bacc()
    score = nc.declare_dram_parameter("score", [BS, NA, NCLS], F32, isOutput=False)
    p_box = nc.declare_dram_parameter("p_box", [BS, NA, 4], F32, isOutput=False)
    anchors = nc.declare_dram_parameter("anchors", [NA, 2], F32, isOutput=False)
    gt_labels = nc.declare_dram_parameter("gt_labels", [BS, NG, 1], I32, isOutput=False)
    gt_box = nc.declare_dram_parameter("gt_box", [BS, NG, 4], F32, isOutput=False)
    maskp = nc.declare_dram_parameter("mask", [BS, NG, 1], F32, isOutput=False)
    out = nc.declare_dram_parameter("out", [BS, NA, 85], F32, isOutput=True)

    with TileContext(nc) as tc:
        with (
            tc.tile_pool(name="big", bufs=1) as big,
            tc.tile_pool(name="sm", bufs=1) as sm,
            tc.tile_pool(name="tmp", bufs=14) as tp,
            tc.tile_pool(name="ps", bufs=6, space="PSUM") as psp,
        ):
            metric = big.tile([128, NA], F32, tag="metric")
            overlaps = big.tile([128, NA], F32, tag="overlaps")
            sel = big.tile([128, NA], F32, tag="sel")
            scr = big.tile([128, NA], F32, tag="scr")
            mgts = big.tile([128, NA], U8, tag="mgts")

            ident = sm.tile([128, 128], F32, tag="ident")
            masks.make_identity(nc, ident[:, :])
            ones1 = sm.tile([1, 128], F32, tag="ones1")
            nc.gpsimd.memset(ones1[:, :], 1.0)
            ones64 = sm.tile([128, 1], F32, tag="ones64")
            nc.gpsimd.memset(ones64[:, :], 1.0)

            # ---- gt-side scalars, packed 2 batches x 64 gts = 128 partitions
            gtc = sm.tile([128, 4], F32, tag="gtc")
            nc.sync.dma_start(out=gtc[:, :], in_=gt_box.rearrange("b n f -> (b n) f"))
            lrow_f = []
            for b in range(2):
                li = sm.tile([1, NG], I32, tag=f"li{b}")
                nc.gpsimd.dma_start(out=li[:, :], in_=gt_labels[b, :, 0:1].rearrange("n o -> o n"))
                lf = sm.tile([1, NG], F32, tag=f"lf{b}")
                nc.vector.tensor_copy(lf[:, :], li[:, :])
                lrow_f.append(lf)
            valid = sm.tile([128, 1], F32, tag="valid")
            nc.sync.dma_start(out=valid[:, :], in_=maskp.rearrange("b n f -> (b n) f"))

            x1s, y1s, x2s, y2s = (gtc[:, k:k + 1] for k in range(4))
            gs = sm.tile([128, 8], F32, tag="gs")
            w1, h1e, atan1, area1, sumx1, sumy1, rh1, rat1 = (gs[:, k:k + 1] for k in range(8))
            nc.any.tensor_tensor(out=w1, in0=x2s, in1=x1s, op=OP.subtract)
            nc.any.tensor_tensor(out=h1e, in0=y2s, in1=y1s, op=OP.subtract)
            nc.any.tensor_scalar(out=h1e, in0=h1e, scalar1=1e-7, scalar2=None, op0=OP.add)
            nc.vector.reciprocal(out=rh1, in_=h1e)
            nc.any.tensor_tensor(out=rat1, in0=w1, in1=rh1, op=OP.mult)
            nc.scalar.activation(atan1, rat1, ACTF.Arctan)
            nc.any.tensor_tensor(out=area1, in0=w1, in1=h1e, op=OP.mult)
            nc.any.tensor_tensor(out=sumx1, in0=x1s, in1=x2s, op=OP.add)
            nc.any.tensor_tensor(out=sumy1, in0=y1s, in1=y2s, op=OP.add)

            # ---- one-hot(labels)^T [80, 64] per batch for the class gather
            iotac80 = sm.tile([80, 1], F32, tag="iotac80")
            nc.gpsimd.iota(iotac80[:, :], pattern=[[0, 1]], base=0, channel_multiplier=1,
                           allow_small_or_imprecise_dtypes=True)
            onehotT = []
            for b in range(2):
                pb = psp.tile([128, W], F32, tag="ps")
                nc.tensor.matmul(pb[0:80, 0:NG], ones1[0:1, 0:80], lrow_f[b][:, :])
                oh = sm.tile([80, NG], F32, tag=f"oht{b}")
                nc.any.tensor_scalar(out=oh[:, :], in0=pb[0:80, 0:NG], scalar1=iotac80[:, :],
                                     scalar2=None, op0=OP.is_equal)
                onehotT.append(oh)

            # ================= stage 1: metric / overlaps / mgts =================
            tc.strict_bb_all_engine_barrier()
            for t in range(NT):
                a0 = t * W
                # broadcast pred-side rows to 128 partitions via PE outer product
                axb = tp.tile([128, W], F32, tag="t")
                ayb = tp.tile([128, W], F32, tag="t")
                for k, dst in ((0, axb), (1, ayb)):
                    arow = tp.tile([1, W], F32, tag="r")
                    nc.gpsimd.dma_start(out=arow[:, :],
                                      in_=anchors[a0:a0 + W, k:k + 1].rearrange("a o -> o a"))
                    pb = psp.tile([128, W], F32, tag="ps")
                    nc.tensor.matmul(pb[0:128, 0:W], ones1[0:1, 0:128], arow[:, :])
                    nc.vector.tensor_copy(dst[:, :], pb[0:128, 0:W])
                coords = []
                for k in range(4):
                    dst = tp.tile([128, W], F32, tag="t")
                    for b in range(2):
                        prow = tp.tile([1, W], F32, tag="r")
                        nc.gpsimd.dma_start(out=prow[:, :],
                                          in_=p_box[b, a0:a0 + W, k:k + 1].rearrange("a o -> o a"))
                        pb = psp.tile([128, W], F32, tag="ps")
                        nc.tensor.matmul(pb[0:64, 0:W], ones1[0:1, 0:64], prow[:, :])
                        nc.vector.tensor_copy(dst[64 * b:64 * b + 64, :], pb[0:64, 0:W])
                    coords.append(dst)
                x1p, y1p, x2p, y2p = coords

                # gathered scores: transpose score chunks then one-hot matmul
                gath = tp.tile([128, W], F32, tag="t")
                for b in range(2):
                    sct = sm.tile([80, W], F32, tag=f"sct{b}")
                    off = 0
                    while off < W:
                        cw = min(128, W - off)
                        st = tp.tile([128, NCLS], F32, tag="st")
                        nc.sync.dma_start(out=st[0:cw, :], in_=score[b, a0 + off:a0 + off + cw, :])
                        pb = psp.tile([128, W], F32, tag="ps")
                        nc.tensor.transpose(pb[0:NCLS, 0:cw], st[0:cw, 0:NCLS], ident[0:cw, 0:cw])
                        nc.vector.tensor_copy(sct[:, off:off + cw], pb[0:NCLS, 0:cw])
                        off += cw
                    pb = psp.tile([128, W], F32, tag="ps")
                    nc.tensor.matmul(pb[0:64, 0:W], onehotT[b][:, :], sct[:, :])
                    nc.vector.tensor_copy(gath[64 * b:64 * b + 64, :], pb[0:64, 0:W])

                def ts(dst, in0, s1, op0, s2=None, op1=OP.bypass):
                    nc.any.tensor_scalar(out=dst, in0=in0, scalar1=s1, scalar2=s2, op0=op0, op1=op1)

                def tt(dst, a, b_, op):
                    nc.any.tensor_tensor(out=dst, in0=a, in1=b_, op=op)

                def tmp():
                    return tp.tile([128, W], F32, tag="t", name="tw")[:, :]

                MG = mgts[:, a0:a0 + W]
                # mask_in_gts & valid
                t1 = tmp(); ts(t1, axb[:, :], x1s, OP.subtract)
                t2 = tmp(); ts(t2, ayb[:, :], y1s, OP.subtract)
                tt(t1, t1, t2, OP.min)
                ts(t2, axb[:, :], x2s, OP.subtract)
                t3 = tmp(); ts(t3, ayb[:, :], y2s, OP.subtract)
                tt(t2, t2, t3, OP.max)
                ts(t2, t2, -1.0, OP.mult)
                tt(t1, t1, t2, OP.min)
                nc.any.tensor_scalar(out=MG, in0=t1, scalar1=1e-9, scalar2=valid[:, :],
                                     op0=OP.is_gt, op1=OP.mult)

                # pred-side derived vectors
                w2 = tmp(); tt(w2, x2p[:, :], x1p[:, :], OP.subtract)
                h2 = tmp(); tt(h2, y2p[:, :], y1p[:, :], OP.subtract)
                ts(h2, h2, 1e-7, OP.add)
                rr = tmp(); nc.vector.reciprocal(out=rr, in_=h2)
                tt(rr, w2, rr, OP.mult)
                at2 = tmp(); nc.scalar.activation(at2, rr, ACTF.Arctan)

                # intersection / union / iou
                a1 = tmp(); ts(a1, x2p[:, :], x2s, OP.min)
                a2 = tmp(); ts(a2, x1p[:, :], x1s, OP.max)
                tt(a1, a1, a2, OP.subtract)
                ts(a1, a1, 0.0, OP.max)
                ts(a2, y2p[:, :], y2s, OP.min)
                b2 = tmp(); ts(b2, y1p[:, :], y1s, OP.max)
                tt(a2, a2, b2, OP.subtract)
                ts(a2, a2, 0.0, OP.max)
                inter = a1; tt(inter, a1, a2, OP.mult)
                u = a2; tt(u, w2, h2, OP.mult)
                nc.any.tensor_scalar(out=u, in0=u, scalar1=area1, scalar2=1e-7, op0=OP.add, op1=OP.add)
                tt(u, u, inter, OP.subtract)
                ru = b2; nc.vector.reciprocal(out=ru, in_=u)
                iou = u; tt(iou, inter, ru, OP.mult)

                # enclosing box diag
                c1 = inter  # reuse
                ts(c1, x2p[:, :], x2s, OP.max)
                c2 = ru
                ts(c2, x1p[:, :], x1s, OP.min)
                tt(c1, c1, c2, OP.subtract)
                tt(c1, c1, c1, OP.mult)
                ts(c2, y2p[:, :], y2s, OP.max)
                d2 = tmp(); ts(d2, y1p[:, :], y1s, OP.min)
                tt(c2, c2, d2, OP.subtract)
                tt(c2, c2, c2, OP.mult)
                tt(c1, c1, c2, OP.add)
                ts(c1, c1, 1e-7, OP.add)
                rc2 = c2; nc.vector.reciprocal(out=rc2, in_=c1)

                # rho2/4
                dx = c1
                tt(dx, x1p[:, :], x2p[:, :], OP.add)
                ts(dx, dx, sumx1, OP.subtract)
                tt(dx, dx, dx, OP.mult)
                dy = d2
                tt(dy, y1p[:, :], y2p[:, :], OP.add)
                ts(dy, dy, sumy1, OP.subtract)
                tt(dy, dy, dy, OP.mult)
                tt(dx, dx, dy, OP.add)
                ts(dx, dx, 0.25, OP.mult)
                rho4 = dx

                # v and v*alpha
                v = dy
                ts(v, at2, atan1, OP.subtract)
                tt(v, v, v, OP.mult)
                ts(v, v, CV, OP.mult)
                va = at2
                tt(va, v, iou, OP.subtract)
                ts(va, va, 1.0 + 1e-7, OP.add)
                rva = w2
                nc.vector.reciprocal(out=rva, in_=va)
                tt(va, v, rva, OP.mult)
                tt(va, va, v, OP.mult)

                # ov = iou - rho4*rc2 - va ; clip; mask
                tt(rho4, rho4, rc2, OP.mult)
                tt(rho4, rho4, va, OP.add)
                OV = overlaps[:, a0:a0 + W]
                tt(OV, iou, rho4, OP.subtract)
                ts(OV, OV, 0.0, OP.max)
                tt(OV, OV, MG, OP.mult)

                # metric = sqrt(gathered) * ov^6
                sq = h2
                nc.scalar.activation(sq, gath[:, :], ACTF.Sqrt)
                o2 = rr
                tt(o2, OV, OV, OP.mult)
                o4 = v
                tt(o4, o2, o2, OP.mult)
                tt(o4, o4, o2, OP.mult)
                ME = metric[:, a0:a0 + W]
                tt(ME, sq, o4, OP.mult)
                if t % 2 == 1:
                    tc.strict_bb_all_engine_barrier()

            # ===== top-k: remove max value-class from a WORKING COPY 10x, then
            # ===== select from the pristine metric against the exact threshold
            tc.strict_bb_all_engine_barrier()
            mjs = sm.tile([128, 10], F32, tag="mjs")
            cnts = sm.tile([128, 10], F32, tag="cnts")
            ngate = sm.tile([128, 1], F32, tag="ngate")
            nmjp1 = sm.tile([128, 1], F32, tag="nmjp1")
            nc.vector.tensor_copy(scr[:, :], metric[:, :])
            for j in range(10):
                mj = mjs[:, j:j + 1]
                nc.vector.tensor_reduce(out=mj, in_=scr[:, :], axis=AX, op=OP.max)
                nc.any.tensor_scalar(out=ngate[:, :], in0=mj, scalar1=0.0, scalar2=-1.0,
                                     op0=OP.is_ge, op1=OP.mult)
                nc.any.tensor_scalar(out=nmjp1[:, :], in0=mj, scalar1=1.0, scalar2=ngate[:, :],
                                     op0=OP.add, op1=OP.mult)
                # sel = (scr == mj), cnts[:,j] = row-sum(sel), in one pass
                nc.vector.scalar_tensor_tensor(out=sel[:, :], in0=scr[:, :], scalar=mj,
                                               in1=scr[:, :], op0=OP.is_equal, op1=OP.bypass,
                                               accum_out=cnts[:, j:j + 1])
                # scr += sel * (-(mj+1)): removed positions become exactly -1
                nc.vector.scalar_tensor_tensor(out=scr[:, :], in0=sel[:, :], scalar=nmjp1[:, :],
                                               in1=scr[:, :], op0=OP.mult, op1=OP.add)

            cum = sm.tile([128, 10], F32, tag="cum")
            nc.vector.tensor_tensor_scan(out=cum[:, :], data0=cnts[:, :], data1=cnts[:, :],
                                         initial=0.0, op0=OP.add, op1=OP.bypass)
            excl = sm.tile([128, 10], F32, tag="excl")
            nc.any.tensor_tensor(out=excl[:, :], in0=cum[:, :], in1=cnts[:, :], op=OP.subtract)
            pick = sm.tile([128, 10], F32, tag="pick")
            nc.any.tensor_scalar(out=pick[:, :], in0=excl[:, :], scalar1=10.0, scalar2=None, op0=OP.is_lt)
            nc.any.tensor_scalar(out=cum[:, :], in0=cum[:, :], scalar1=10.0, scalar2=None, op0=OP.is_ge)
            nc.any.tensor_tensor(out=pick[:, :], in0=pick[:, :], in1=cum[:, :], op=OP.mult)
            Tv = sm.tile([128, 1], F32, tag="Tv")
            nc.any.tensor_tensor(out=cum[:, :], in0=pick[:, :], in1=mjs[:, :], op=OP.mult)
            nc.vector.tensor_reduce(out=Tv[:, :], in_=cum[:, :], axis=AX, op=OP.add)
            mallow = sm.tile([128, 1], F32, tag="mallow")
            nc.any.tensor_tensor(out=cum[:, :], in0=pick[:, :], in1=excl[:, :], op=OP.mult)
            nc.vector.tensor_reduce(out=mallow[:, :], in_=cum[:, :], axis=AX, op=OP.add)
            nc.any.tensor_scalar(out=mallow[:, :], in0=mallow[:, :], scalar1=-1.0, scalar2=10.0,
                                 op0=OP.mult, op1=OP.add)

            # exact selection from pristine metric: v > T, or v == T with tie-rank <= m
            nc.any.tensor_scalar(out=sel[:, :], in0=metric[:, :], scalar1=Tv[:, :], scalar2=None,
                                 op0=OP.is_equal)
            nc.vector.tensor_tensor_scan(out=scr[:, :], data0=sel[:, :], data1=sel[:, :],
                                         initial=0.0, op0=OP.add, op1=OP.bypass)
            nc.vector.scalar_tensor_tensor(out=scr[:, :], in0=scr[:, :], scalar=mallow[:, :],
                                           in1=sel[:, :], op0=OP.is_le, op1=OP.mult)
            nc.vector.scalar_tensor_tensor(out=sel[:, :], in0=metric[:, :], scalar=Tv[:, :],
                                           in1=scr[:, :], op0=OP.is_gt, op1=OP.add)
            nc.any.tensor_tensor(out=sel[:, :], in0=sel[:, :], in1=mgts[:, :], op=OP.mult)

            # ====== stage 2: per-anchor-chunk resolution in transposed layout ======
            tc.strict_bb_all_engine_barrier()
            iota64mb = sm.tile([128, 64], F32, tag="i64")
            nc.gpsimd.iota(iota64mb[:, :], pattern=[[1, 64]], base=-16384, channel_multiplier=0,
                           allow_small_or_imprecise_dtypes=True)
            iota80b = sm.tile([128, 80], F32, tag="i80")
            nc.gpsimd.iota(iota80b[:, :], pattern=[[1, 80]], base=0, channel_multiplier=0,
                           allow_small_or_imprecise_dtypes=True)
            # broadcast gt tables along free dim: cols 0:64 = batch0, 64:128 = batch1
            gtabs = []
            for k in range(4):
                gtab = sm.tile([128, 128], F32, tag=f"gtab{k}")
                for b in range(2):
                    grow = sm.tile([1, NG], F32, tag="grow")
                    nc.gpsimd.dma_start(out=grow[:, :],
                                      in_=gt_box[b, :, k:k + 1].rearrange("n o -> o n"))
                    pb = psp.tile([128, W], F32, tag="ps")
                    nc.tensor.matmul(pb[0:128, 0:NG], ones1[0:1, 0:128], grow[:, :])
                    nc.vector.tensor_copy(gtab[:, 64 * b:64 * b + 64], pb[0:128, 0:NG])
                gtabs.append(gtab)
            lblt = sm.tile([128, 128], F32, tag="lblt")
            for b in range(2):
                pb = psp.tile([128, W], F32, tag="ps")
                nc.tensor.matmul(pb[0:128, 0:NG], ones1[0:1, 0:128], lrow_f[b][:, :])
                nc.vector.tensor_copy(lblt[:, 64 * b:64 * b + 64], pb[0:128, 0:NG])

            fg2all = sm.tile([128, 2 * NCH], F32, tag="fg2all")
            lblall = sm.tile([128, 2 * NCH], F32, tag="lblall")

            for c in range(NCH):
                a0 = c * CH
                cw = min(CH, NA - a0)
                mpT = tp.tile([128, 128], F32, tag="t")
                ovT = tp.tile([128, 128], F32, tag="t")
                for b in range(2):
                    idb = ident[64 * b:64 * b + 64, 64 * b:64 * b + 64]
                    pb = psp.tile([128, W], F32, tag="ps")
                    nc.tensor.transpose(pb[0:cw, 0:64], sel[64 * b:64 * b + 64, a0:a0 + cw], idb)
                    nc.vector.tensor_copy(mpT[0:cw, 64 * b:64 * b + 64], pb[0:cw, 0:64])
                    pb2 = psp.tile([128, W], F32, tag="ps")
                    nc.tensor.transpose(pb2[0:cw, 0:64], overlaps[64 * b:64 * b + 64, a0:a0 + cw], idb)
                    nc.vector.tensor_copy(ovT[0:cw, 64 * b:64 * b + 64], pb2[0:cw, 0:64])
                for b in range(2):
                    S = slice(64 * b, 64 * b + 64)
                    fg = tp.tile([128, 1], F32, tag="s1")
                    nc.vector.tensor_reduce(out=fg[0:cw, :], in_=mpT[0:cw, S], axis=AX, op=OP.add)
                    rmx = tp.tile([128, 1], F32, tag="s1")
                    nc.vector.tensor_reduce(out=rmx[0:cw, :], in_=ovT[0:cw, S], axis=AX, op=OP.max)
                    e = tp.tile([128, 64], F32, tag="t")
                    nc.any.tensor_scalar(out=e[0:cw, :], in0=ovT[0:cw, S], scalar1=rmx[0:cw, :],
                                         scalar2=None, op0=OP.is_equal)
                    nc.any.tensor_tensor(out=e[0:cw, :], in0=e[0:cw, :], in1=iota64mb[0:cw, :], op=OP.mult)
                    rmn = tp.tile([128, 1], F32, tag="s1")
                    nc.vector.tensor_reduce(out=rmn[0:cw, :], in_=e[0:cw, :], axis=AX, op=OP.min)
                    oh = tp.tile([128, 64], F32, tag="t")
                    nc.any.tensor_scalar(out=oh[0:cw, :], in0=e[0:cw, :], scalar1=rmn[0:cw, :],
                                         scalar2=None, op0=OP.is_equal)
                    g = tp.tile([128, 1], F32, tag="s1")
                    nc.any.tensor_scalar(out=g[0:cw, :], in0=fg[0:cw, :], scalar1=1.0, scalar2=None,
                                         op0=OP.is_gt)
                    nc.any.tensor_tensor(out=oh[0:cw, :], in0=oh[0:cw, :], in1=mpT[0:cw, S], op=OP.subtract)
                    nc.any.tensor_scalar(out=oh[0:cw, :], in0=oh[0:cw, :], scalar1=g[0:cw, :],
                                         scalar2=None, op0=OP.mult)
                    nc.any.tensor_tensor(out=mpT[0:cw, S], in0=mpT[0:cw, S], in1=oh[0:cw, :], op=OP.add)
                    nc.vector.tensor_reduce(out=fg2all[0:cw, 2 * c + b:2 * c + b + 1],
                                            in_=mpT[0:cw, S], axis=AX, op=OP.add)
                    # gt_idx one-hot (argmax of resolved mask over gts)
                    nc.vector.tensor_reduce(out=rmx[0:cw, :], in_=mpT[0:cw, S], axis=AX, op=OP.max)
                    nc.any.tensor_scalar(out=e[0:cw, :], in0=mpT[0:cw, S], scalar1=rmx[0:cw, :],
                                         scalar2=None, op0=OP.is_equal)
                    nc.any.tensor_tensor(out=e[0:cw, :], in0=e[0:cw, :], in1=iota64mb[0:cw, :], op=OP.mult)
                    nc.vector.tensor_reduce(out=rmn[0:cw, :], in_=e[0:cw, :], axis=AX, op=OP.min)
                    nc.any.tensor_scalar(out=e[0:cw, :], in0=e[0:cw, :], scalar1=rmn[0:cw, :],
                                         scalar2=None, op0=OP.is_equal)
                    # gather label + bbox via one-hot
                    ob = tp.tile([128, 64], F32, tag="t")
                    nc.any.tensor_tensor(out=ob[0:cw, :], in0=e[0:cw, :], in1=lblt[0:cw, S], op=OP.mult)
                    nc.vector.tensor_reduce(out=lblall[0:cw, 2 * c + b:2 * c + b + 1],
                                            in_=ob[0:cw, :], axis=AX, op=OP.add)
                    outsb = tp.tile([128, 5], F32, tag="o5")
                    for k in range(4):
                        nc.any.tensor_tensor(out=ob[0:cw, :], in0=e[0:cw, :], in1=gtabs[k][0:cw, S],
                                             op=OP.mult)
                        nc.vector.tensor_reduce(out=outsb[0:cw, k:k + 1], in_=ob[0:cw, :], axis=AX,
                                                op=OP.add)
                    nc.any.tensor_scalar(out=outsb[0:cw, 4:5], in0=fg2all[0:cw, 2 * c + b:2 * c + b + 1],
                                         scalar1=0.0, scalar2=None, op0=OP.is_gt)
                    nc.sync.dma_start(out=out[b, a0:a0 + cw, 0:4], in_=outsb[0:cw, 0:4])
                    nc.sync.dma_start(out=out[b, a0:a0 + cw, 84:85], in_=outsb[0:cw, 4:5])
                # write resolved mask back to gt-layout
                for b in range(2):
                    pb = psp.tile([128, W], F32, tag="ps")
                    nc.tensor.transpose(pb[0:64, 0:cw], mpT[0:cw, 64 * b:64 * b + 64],
                                        ident[0:cw, 0:cw])
                    nc.vector.tensor_copy(sel[64 * b:64 * b + 64, a0:a0 + cw], pb[0:64, 0:cw])
                if c % 4 == 3:
                    tc.strict_bb_all_engine_barrier()

            # ====== stage 3: normalization factors and per-anchor score scale ======
            tc.strict_bb_all_engine_barrier()
            nc.any.tensor_tensor(out=scr[:, :], in0=metric[:, :], in1=sel[:, :], op=OP.mult)
            posm = sm.tile([128, 1], F32, tag="posm")
            nc.vector.tensor_reduce(out=posm[:, :], in_=scr[:, :], axis=AX, op=OP.max)
            nc.any.tensor_tensor(out=overlaps[:, :], in0=overlaps[:, :], in1=sel[:, :], op=OP.mult)
            poso = sm.tile([128, 1], F32, tag="poso")
            nc.vector.tensor_reduce(out=poso[:, :], in_=overlaps[:, :], axis=AX, op=OP.max)
            nc.any.tensor_scalar(out=posm[:, :], in0=posm[:, :], scalar1=1e-9, scalar2=None, op0=OP.add)
            rpm = sm.tile([128, 1], F32, tag="rpm")
            nc.vector.reciprocal(out=rpm[:, :], in_=posm[:, :])
            nc.any.tensor_tensor(out=rpm[:, :], in0=poso[:, :], in1=rpm[:, :], op=OP.mult)
            nc.any.tensor_scalar(out=scr[:, :], in0=scr[:, :], scalar1=rpm[:, :], scalar2=None,
                                 op0=OP.mult)

            # ====== stage 4: final scores ======
            for c in range(NCH):
                a0 = c * CH
                cw = min(CH, NA - a0)
                for b in range(2):
                    spb = psp.tile([128, W], F32, tag="ps")
                    nc.tensor.matmul(spb[0:cw, 0:1], scr[64 * b:64 * b + 64, a0:a0 + cw],
                                     ones64[64 * b:64 * b + 64, :])
                    ssb = tp.tile([128, 1], F32, tag="s1")
                    nc.vector.tensor_copy(ssb[0:cw, :], spb[0:cw, 0:1])
                    f2 = tp.tile([128, 1], F32, tag="s1")
                    nc.any.tensor_scalar(out=f2[0:cw, :], in0=fg2all[0:cw, 2 * c + b:2 * c + b + 1],
                                         scalar1=0.0, scalar2=None, op0=OP.is_gt)
                    osc = tp.tile([128, 80], F32, tag="t")
                    nc.any.tensor_scalar(out=osc[0:cw, :], in0=iota80b[0:cw, :],
                                         scalar1=lblall[0:cw, 2 * c + b:2 * c + b + 1],
                                         scalar2=f2[0:cw, :], op0=OP.is_equal, op1=OP.mult)
                    nc.any.tensor_scalar(out=osc[0:cw, :], in0=osc[0:cw, :], scalar1=ssb[0:cw, :],
                                         scalar2=None, op0=OP.mult)
                    nc.sync.dma_start(out=out[b, a0:a0 + cw, 4:84], in_=osc[0:cw, :])
                if c % 8 == 7:
                    tc.strict_bb_all_engine_barrier()
    nc.compile()
    return nc


_NC_CACHE = None


def kernel(score, p_box, anchors, gt_labels, gt_box, mask):
    global _NC_CACHE
    if _NC_CACHE is None:
        _NC_CACHE = build()
    nc = _NC_CACHE
    in_maps = []
    for c in range(NCORES):
        s = slice(c * BS, (c + 1) * BS)
        in_maps.append({
            "score": np.ascontiguousarray(score[s], dtype=np.float32),
            "p_box": np.ascontiguousarray(p_box[s], dtype=np.float32),
            "anchors": np.ascontiguousarray(anchors, dtype=np.float32),
            "gt_labels": np.ascontiguousarray(gt_labels[s], dtype=np.int32),
            "gt_box": np.ascontiguousarray(gt_box[s], dtype=np.float32),
            "mask": np.ascontiguousarray(mask[s], dtype=np.float32),
        })
    res = run_bass_kernel_spmd(nc, in_maps, list(range(NCORES)))
    outs = [np.asarray(res.results[c]["out"]) for c in range(NCORES)]
    full = np.concatenate(outs, axis=0)  # [16, 8400, 85]
    tb = np.ascontiguousarray(full[..., 0:4], dtype=np.float32)
    tsc = np.ascontiguousarray(full[..., 4:84], dtype=np.float32)
    fg = full[..., 84] > 0.5
    return tb, tsc, fg
